# revision 20
# baseline (speedup 1.0000x reference)
"""Trainium2 Bass kernel for CustomEmbeddings (embedding lookup + masked MLP).

Computation (reference):
    emb = emb_table[input_ids]                    # [B, S, D]
    mask = input_ids >= 32000
    h = relu(emb @ w1 + b1); mlp = h @ w2 + b2
    out = where(mask, mlp, emb)

Strategy (8 NeuronCores, SPMD — same program, per-core data):
  - Token-parallel over id-sorted chunks: the host stable-sorts the 16384
    token positions by id and deals 2048 consecutive tokens to each core.
    Sorting clusters duplicate ids on one core, so the union of each
    core's unique rows is (almost exactly) the global unique set: the host
    ships core c a PACKED fp16 table of just its ~1650 unique rows
    (10.7 MB vs 53.7 MB for the fp32 vocab-range slab), and the device
    expands rows -> per-token output with a real indirect gather
    (token -> packed-row indices are scattered, data-dependent values).
    The host inverse-permutes the concatenated outputs (unsharding).
  - Everything moves as fp16 (table shard, gathered output, MLP weights):
    the harness gate is rel_err < 2e-2 and fp16 costs ~5e-4.  Per-core
    I/O drops ~99 MB -> ~35.4 MB, which sets the HW execution span (the
    per-token program work measures far below the I/O staging time:
    rep-amplifying the gather x5 and the MLP x21 moved per-call wall
    time by <2 ms, while declared I/O bytes move it at ~11.5 GB/s).
  - The masked-token MLP (~51 tokens, one 128-row chunk) is weight-sharded
    8 ways: core c holds pre-shuffled fp16 w1[:, c*800:(c+1)*800] and
    w2[c*800:(c+1)*800, :] (single big DMA each, preloaded in SBUF), and
    computes its partial mlp_out = relu(emb@w1_c + b1_c) @ w2_c in fp16 on
    the PE.  The 8 partials are summed on the host (+ b2) and scattered
    into the masked positions.
"""

import sys

if "/opt/trn_rl_repo" not in sys.path:
    sys.path.insert(0, "/opt/trn_rl_repo")

import numpy as np

from concourse import bacc, bass, mybir
import concourse.tile as tile
from concourse.bass_utils import run_bass_kernel_spmd
from concourse.masks import make_identity

P = 128
VOCAB = 32100
DIM = 3200
HID = 6400
NEW_START = 32000
N_CORES = 8
SHARD_HID = HID // N_CORES          # 800
# mlp_tab row count is data-dependent (distinct masked ids, 16-padded)
N_K_TILES = DIM // P                # 25
TOK_PER_CORE = 2048                 # 8*2048 tokens / 8 cores
N_T_CHUNKS = TOK_PER_CORE // P      # 16


def cdiv(a, b):
    return (a + b - 1) // b


N_HB = cdiv(SHARD_HID, P)           # 7 h-column blocks (6 full + 32)
HID_PAD = N_HB * P                  # 896

# Testing hooks: repeat a section this many times (same data, same outputs)
# so HW wall-clock scaling can separate device time from dispatch overhead.
# Always 1 in normal use.
GATHER_REPS = 1
MLP_REPS = 1


def build_program(n_mlp_chunks: int, t_cap: int, m_cap: int) -> bass.Bass:
    f16 = mybir.dt.float16
    f32 = mybir.dt.float32
    i32 = mybir.dt.int32

    # Bacc (not plain Bass): its finalize() runs the wait-legalization passes
    # (move_matmul_waits_to_ldweights / generate_event_semaphores) that split
    # multi-wait instructions the TRN2 ISA encodings cannot carry.
    nc = bacc.Bacc("TRN2")
    ids_t = nc.declare_dram_parameter("ids_t", [P, N_T_CHUNKS], i32, isOutput=False)
    mlp_ids = nc.declare_dram_parameter(
        "mlp_ids", [P, n_mlp_chunks], i32, isOutput=False
    )
    tshard = nc.declare_dram_parameter("tshard", [t_cap, DIM], f16, isOutput=False)
    mlp_tab = nc.declare_dram_parameter(
        "mlp_tab", [m_cap, DIM], f16, isOutput=False
    )
    # w1s[p, k*800+j]   = w1[k*128+p, c*800+j]          (k < 25)
    # w2s[p, k2*3200+j] = w2[c*800 + k2*128+p, j]       (k2 < 6, full blocks)
    # w2t[p, j]         = w2[c*800 + 768 + p, j]        (32-row tail block)
    w1s = nc.declare_dram_parameter(
        "w1s", [P, N_K_TILES * SHARD_HID], f16, isOutput=False
    )
    b1s = nc.declare_dram_parameter("b1s", [1, SHARD_HID], f16, isOutput=False)
    w2s = nc.declare_dram_parameter("w2s", [P, (N_HB - 1) * DIM], f16, isOutput=False)
    w2t = nc.declare_dram_parameter("w2t", [SHARD_HID - (N_HB - 1) * P, DIM], f16, isOutput=False)
    out_main = nc.declare_dram_parameter(
        "out_main", [TOK_PER_CORE, DIM], f16, isOutput=True
    )
    mlp_part = nc.declare_dram_parameter(
        "mlp_part", [n_mlp_chunks * P, DIM], f16, isOutput=True
    )

    with tile.TileContext(nc) as tc:
        with (
            tc.tile_pool(name="const", bufs=1) as consts,
            tc.tile_pool(name="gpool", bufs=3) as gpool,
            tc.tile_pool(name="mpool", bufs=1) as mpool,
            tc.tile_pool(name="psA", bufs=2, space="PSUM") as psA,
            tc.tile_pool(name="psH", bufs=1, space="PSUM") as psH,
            tc.tile_pool(name="psO", bufs=1, space="PSUM") as psO,
        ):
            ones_row = consts.tile([1, P], f16)
            nc.gpsimd.memset(ones_row[:], 1.0)
            identity = consts.tile([P, P], f16)
            make_identity(nc, identity[:])
            # Priming transpose: the PE transpose lowers to a pure LW
            # instruction that supports only ONE sync wait.  This op makes PE
            # observe the Pool semaphore (identity/ones memsets), so later
            # transposes only wait on their data input.
            prime = psA.tile([P, P], f16, space="PSUM", tag="tp")
            nc.tensor.transpose(out=prime[:], in_=identity[:], identity=identity[:])

            idx_sb = consts.tile([P, N_T_CHUNKS], i32)
            nc.sync.dma_start(out=idx_sb[:], in_=ids_t[:])
            midx_sb = consts.tile([P, n_mlp_chunks], i32)
            nc.sync.dma_start(out=midx_sb[:], in_=mlp_ids[:])
            b1_sb = consts.tile([1, SHARD_HID], f16)
            nc.sync.dma_start(out=b1_sb[:], in_=b1s[:])
            # Preloaded fp16 weights: one big DMA each, on the Activation
            # HWDGE queue so they stream in parallel with the SP-queue
            # gather writes below.
            w1_sb = consts.tile([P, N_K_TILES * SHARD_HID], f16)
            nc.scalar.dma_start(out=w1_sb[:], in_=w1s[:])
            w2_sb = consts.tile([P, (N_HB - 1) * DIM], f16)
            nc.scalar.dma_start(out=w2_sb[:], in_=w2s[:])
            TAIL = SHARD_HID - (N_HB - 1) * P  # 32
            w2t_sb = consts.tile([TAIL, DIM], f16)
            nc.scalar.dma_start(out=w2t_sb[:], in_=w2t[:])

            # ---------------- masked-token MLP (small; overlaps with gather) ----
            for j in [j for _ in range(MLP_REPS) for j in range(n_mlp_chunks)]:
                memb = mpool.tile([P, DIM], f16, tag="memb")
                nc.gpsimd.indirect_dma_start(
                    out=memb[:],
                    out_offset=None,
                    in_=mlp_tab[:],
                    in_offset=bass.IndirectOffsetOnAxis(
                        ap=midx_sb[:, j : j + 1], axis=0
                    ),
                )
                # embT[p, k*P + t] = memb[t, k*P + p]
                embT = mpool.tile([P, DIM], f16, tag="embT")
                for k in range(N_K_TILES):
                    tp = psA.tile([P, P], f16, space="PSUM", tag="tp")
                    nc.tensor.transpose(
                        out=tp[:], in_=memb[:, k * P : (k + 1) * P], identity=identity[:]
                    )
                    nc.vector.tensor_copy(out=embT[:, k * P : (k + 1) * P], in_=tp[:])

                # L1: h = relu(emb @ w1s + b1s), h in [tokens, SHARD_HID]
                hps = psH.tile([P, SHARD_HID], f32, space="PSUM", tag="hps")
                for k in range(N_K_TILES):
                    for n0 in range(0, SHARD_HID, 512):
                        n1 = min(n0 + 512, SHARD_HID)
                        nc.tensor.matmul(
                            hps[:, n0:n1],
                            lhsT=embT[:, k * P : (k + 1) * P],
                            rhs=w1_sb[:, k * SHARD_HID + n0 : k * SHARD_HID + n1],
                            start=(k == 0),
                            stop=False,
                        )
                # bias add as rank-1 update: ones[tokens] x b1[cols]
                for n0 in range(0, SHARD_HID, 512):
                    n1 = min(n0 + 512, SHARD_HID)
                    nc.tensor.matmul(
                        hps[:, n0:n1],
                        lhsT=ones_row[:1, :],
                        rhs=b1_sb[:1, n0:n1],
                        start=False,
                        stop=True,
                    )
                h_sb = mpool.tile([P, SHARD_HID], f16, tag="h_sb")
                nc.scalar.activation(
                    out=h_sb[:],
                    in_=hps[:],
                    func=mybir.ActivationFunctionType.Relu,
                )

                # hT[p, k2*P + t] = h[t, k2*P + p]; last block is 32 wide
                hT = mpool.tile([P, HID_PAD], f16, tag="hT")
                for k2 in range(N_HB):
                    bs = min(P, SHARD_HID - k2 * P)
                    tp2 = psA.tile([P, P], f16, space="PSUM", tag="tp")
                    nc.tensor.transpose(
                        out=tp2[:bs, :],
                        in_=h_sb[:, k2 * P : k2 * P + bs],
                        identity=identity[:],
                    )
                    nc.vector.tensor_copy(
                        out=hT[:bs, k2 * P : (k2 + 1) * P], in_=tp2[:bs, :]
                    )

                # L2 partial: mlp_part = h_c @ w2_c, computed in two column halves
                HALF = DIM // 2  # 1600 -> 4 PSUM banks
                for hh in range(2):
                    c0 = hh * HALF
                    ops = psO.tile([P, HALF], f32, space="PSUM", tag="ops")
                    for k2 in range(N_HB):
                        bs = min(P, SHARD_HID - k2 * P)
                        for n0 in range(0, HALF, 512):
                            n1 = min(n0 + 512, HALF)
                            rhs = (
                                w2_sb[:, k2 * DIM + c0 + n0 : k2 * DIM + c0 + n1]
                                if k2 < N_HB - 1
                                else w2t_sb[:, c0 + n0 : c0 + n1]
                            )
                            nc.tensor.matmul(
                                ops[:, n0:n1],
                                lhsT=hT[:bs, k2 * P : (k2 + 1) * P],
                                rhs=rhs,
                                start=(k2 == 0),
                                stop=(k2 == N_HB - 1),
                            )
                    ocp = mpool.tile([P, HALF], f16, tag="ocp")
                    nc.vector.tensor_copy(out=ocp[:], in_=ops[:])
                    nc.sync.dma_start(
                        out=mlp_part[j * P : (j + 1) * P, c0 : c0 + HALF], in_=ocp[:]
                    )

            # ---------------- main gather: 2048 token rows/core ------------------
            for t in [t for _ in range(GATHER_REPS) for t in range(N_T_CHUNKS)]:
                g = gpool.tile([P, DIM], f16, tag="g")
                nc.gpsimd.indirect_dma_start(
                    out=g[:],
                    out_offset=None,
                    in_=tshard[:],
                    in_offset=bass.IndirectOffsetOnAxis(
                        ap=idx_sb[:, t : t + 1], axis=0
                    ),
                )
                nc.sync.dma_start(out=out_main[t * P : (t + 1) * P, :], in_=g[:])

    if not nc.is_finalized():
        nc.finalize()
    return nc


def _wrap(ids, n_chunks):
    """[n_chunks*P] -> [P, n_chunks] with element [p, c] = ids[c*P + p]."""
    return np.ascontiguousarray(ids.reshape(n_chunks, P).T.astype(np.int32))


def _prepare(inputs):
    """Host-side sharding. Returns (n_mlp_chunks, t_cap, in_maps, ctx)."""
    ids = np.asarray(inputs["input_ids"])
    table = np.asarray(inputs["emb_table"], dtype=np.float32)
    w1 = np.asarray(inputs["w1"], dtype=np.float32)
    b1 = np.asarray(inputs["b1"], dtype=np.float32)
    w2 = np.asarray(inputs["w2"], dtype=np.float32)
    b2 = np.asarray(inputs["b2"], dtype=np.float32)

    B, S = ids.shape
    ids_flat = ids.reshape(-1).astype(np.int64)
    N = ids_flat.size
    assert N == N_CORES * TOK_PER_CORE

    table16 = table.astype(np.float16)

    # --- masked tokens (global; same for every core) ---
    mask = ids_flat >= NEW_START
    masked_pos = np.nonzero(mask)[0]
    K = int(masked_pos.size)
    n_mlp_chunks = max(1, cdiv(K, P))
    # pack only the DISTINCT masked ids (typically ~50 of the 100 new rows)
    m_uniq = np.unique(ids_flat[masked_pos]) if K > 0 else np.zeros(1, np.int64)
    m_cap = cdiv(max(int(m_uniq.size), 1), 16) * 16
    mids = np.zeros(n_mlp_chunks * P, dtype=np.int64)
    if K > 0:
        mids[:K] = np.searchsorted(m_uniq, ids_flat[masked_pos])
    mlp_ids_t = _wrap(mids, n_mlp_chunks)
    mlp_tab = np.zeros((m_cap, DIM), dtype=np.float16)
    mlp_tab[: m_uniq.size] = table16[m_uniq]

    # --- id-sorted token routing: duplicates cluster, so each core's packed
    # unique-row table is ~1/8 of the global unique set ---
    order = np.argsort(ids_flat, kind="stable")
    uniq_per_core, loc_per_core = [], []
    for c in range(N_CORES):
        sid = ids_flat[order[c * TOK_PER_CORE : (c + 1) * TOK_PER_CORE]]
        uniq_c, loc = np.unique(sid, return_inverse=True)
        uniq_per_core.append(uniq_c)
        loc_per_core.append(loc)
    # 16-row granularity: the gather only needs idx values < U_c, not a
    # 128-multiple table; finer padding saves shipped bytes.
    t_cap = cdiv(max(int(u.size) for u in uniq_per_core), 16) * 16

    in_maps = []
    n_full = N_HB - 1  # 6 full 128-row w2 blocks; 32-row tail ships separately
    for c in range(N_CORES):
        uniq_c = uniq_per_core[c]
        tshard = np.zeros((t_cap, DIM), dtype=np.float16)
        tshard[: uniq_c.size] = table16[uniq_c]
        w1c = w1[:, c * SHARD_HID : (c + 1) * SHARD_HID]
        w1s = np.ascontiguousarray(
            w1c.reshape(N_K_TILES, P, SHARD_HID).transpose(1, 0, 2).reshape(P, -1)
        ).astype(np.float16)
        w2c = w2[c * SHARD_HID : (c + 1) * SHARD_HID, :]
        w2s = np.ascontiguousarray(
            w2c[: n_full * P].reshape(n_full, P, DIM).transpose(1, 0, 2).reshape(P, -1)
        ).astype(np.float16)
        w2tail = np.ascontiguousarray(w2c[n_full * P :]).astype(np.float16)
        in_maps.append(
            {
                "ids_t": _wrap(loc_per_core[c].astype(np.int64), N_T_CHUNKS),
                "mlp_ids": mlp_ids_t,
                "tshard": tshard,
                "mlp_tab": mlp_tab,
                "w1s": w1s,
                "b1s": b1[c * SHARD_HID : (c + 1) * SHARD_HID]
                .astype(np.float16)
                .reshape(1, SHARD_HID),
                "w2s": w2s,
                "w2t": w2tail,
            }
        )
    ctx = dict(B=B, S=S, N=N, masked_pos=masked_pos, K=K, b2=b2, order=order)
    return n_mlp_chunks, t_cap, m_cap, in_maps, ctx


def _finish(results, ctx):
    rows = np.concatenate(
        [results[c]["out_main"] for c in range(N_CORES)]
    )  # [N, DIM] f16, in id-sorted token order
    out = np.empty((ctx["N"], DIM), dtype=np.float32)
    out[ctx["order"]] = rows.astype(np.float32)
    K = ctx["K"]
    if K > 0:
        mlp = results[0]["mlp_part"].astype(np.float32)
        for c in range(1, N_CORES):
            mlp = mlp + results[c]["mlp_part"].astype(np.float32)
        mlp += ctx["b2"][None, :]
        out[ctx["masked_pos"]] = mlp[:K]
    return out.reshape(ctx["B"], ctx["S"], DIM)


def kernel(**inputs) -> np.ndarray:
    n_mlp_chunks, t_cap, m_cap, in_maps, ctx = _prepare(inputs)
    nc = build_program(n_mlp_chunks, t_cap, m_cap)
    res = run_bass_kernel_spmd(nc, in_maps, list(range(N_CORES))).results
    return _finish(res, ctx)
